# revision 28
# baseline (speedup 1.0000x reference)
"""Trainium2 Bass kernel for 7x7 sliding-window self-similarity attention.

out[b,c,h,w] = sum_j softmax_j(x[h,w] * x[h+dh,w+dw]) * x[h+dh,w+dw]
over the 7x7 neighborhood (zero padding, pad=3).

Sharding: B*C = 256 independent 128x128 images, 32 images per core on 8
NeuronCores (pure data parallel, no collectives).

Per-core layout: partition p = rowblock(0..3)*32 + image(0..31); each
partition holds a 44-row x 140-col zero-padded fp32 slab (6160 contiguous
floats), so every 7x7 shift is a flat offset view. Elementwise ops run on
fully contiguous 1D runs spanning the pad columns (finite garbage there,
never read).

Score symmetry: e_{-d}[i] == e_d[i-d]; only 25 canonical score tiles are
computed on an extended halo run; mirrored contributions are views.

Numerator trick: sum_d e_d[i]*x[i+d] = (sum of t_d = e_d*s_d views)/x[i]
(s_d is the score itself), so both the +d and -d numerator contributions
are views of one t tile; the final division by x cancels exactly:
out = acc_t / (x * sum_e).

Engines: DVE does score/t products and the acc_t chain; ACT does exp;
TensorE accumulates sum_e into PSUM via fp32 (LOW_HIGH) identity
matmuls on its own SBUF ports; GpSimd stays idle (it shares DVE's
second SBUF read port - concurrency measured 3x slower on both).
"""

import numpy as np

import concourse.bacc as bacc
import concourse.bass as bass  # noqa: F401
import concourse.tile as tile
from concourse import mybir
from concourse.bass_utils import run_bass_kernel_spmd

N_CORES = 8
F32 = mybir.dt.float32
MULT = mybir.AluOpType.mult
ADD = mybir.AluOpType.add

B, C, H, W = 4, 64, 128, 128
N_IMG_TOTAL = B * C
IMG_PER_CORE = N_IMG_TOTAL // N_CORES  # 32
RB_N = 4
PAD = 6
MM_CHUNK = 512                # one PSUM bank of fp32


def canonical_offsets():
    canon = [(0, 0)]
    canon += [(0, dj) for dj in range(1, 4)]
    canon += [(di, dj) for di in range(1, 4) for dj in range(-3, 4)]
    return canon


def view2d(ap, off, rows, cols, stride):
    """Strided [rows, cols] view at element offset `off` of a flat [P, L] AP."""
    a = ap.copy()
    pair_t = type(a.ap)
    part = list(a.ap)[0]
    a.ap = pair_t([list(part), [stride, rows], [1, cols]])
    a.offset = a.offset + off
    return a


def build_nc(n_img=IMG_PER_CORE, h=H, w=W):
    br = h // RB_N               # 32
    wp = w + 2 * PAD             # 140
    slab = br + 2 * PAD          # 44
    P = n_img * RB_N             # 128

    nx = slab * wp               # 6160
    le = (br + 6) * wp + 8       # 5328 extended run
    soff = 3 * wp - 4
    la = br * wp                 # 4480 full-width run
    lc = br * w                  # 4096 compact output
    t0_off = 3 * wp + 4
    xq_off = 6 * wp
    mm_chunk = min(MM_CHUNK, lc)
    n_chunks = lc // mm_chunk
    rpc = mm_chunk // w

    nc = bacc.Bacc("TRN2", target_bir_lowering=False, debug=False)
    x_in = nc.dram_tensor("x", [P, nx], F32, kind="ExternalInput")
    id_in = nc.dram_tensor("ident", [P, P], F32, kind="ExternalInput")
    y_out = nc.dram_tensor("y", [P, lc], F32, kind="ExternalOutput")

    canon = canonical_offsets()
    n_views = 2 * len(canon) - 1  # 49

    with tile.TileContext(nc) as tc:
        with (
            tc.tile_pool(name="big", bufs=1) as big,
            tc.tile_pool(name="sp", bufs=2) as spool,
            tc.tile_pool(name="ep", bufs=3) as epool,
            tc.tile_pool(name="tp", bufs=1) as tpool,
            tc.tile_pool(name="fin", bufs=1) as fin,
            tc.tile_pool(name="ps", bufs=1, space="PSUM") as ps,
        ):
            x = big.tile([P, nx], F32, tag="x")
            ident = big.tile([P, P], F32, tag="id")
            acc = big.tile([P, la], F32, tag="acc")
            psum = ps.tile([P, lc], F32, tag="sum")

            nrd = 6 * wp + la + 3 * wp + 3 + 1  # last element ever read
            nc.sync.dma_start(out=x[:, :nrd], in_=x_in[:, :nrd])
            nc.sync.dma_start(out=ident[:], in_=id_in[:])

            vidx = 0
            aidx = 0

            def emit_score(di, dj):
                # s_d = x * shift(x, d) on the minimal run [t0-df, t0+la)
                df = di * wp + dj
                lo = t0_off - df
                ln = la + df
                s = spool.tile([P, le], F32, tag="s")
                e = epool.tile([P, le], F32, tag="e")
                sv = s[:, lo:lo + ln]
                if df == 0:
                    nc.scalar.activation(
                        out=sv, in_=x[:, soff + lo:soff + lo + ln],
                        func=mybir.ActivationFunctionType.Square,
                    )
                else:
                    nc.vector.tensor_tensor(
                        out=sv,
                        in0=x[:, soff + lo:soff + lo + ln],
                        in1=x[:, soff + lo + df:soff + lo + df + ln],
                        op=MULT,
                    )
                ev = e[:, lo:lo + ln]
                nc.scalar.activation(
                    out=ev, in_=sv, func=mybir.ActivationFunctionType.Exp
                )
                return s, e, sv, ev, df, lo, ln

            pending = emit_score(*canon[0])
            for k in range(len(canon)):
                s, e, sv, ev, df, lo, ln = pending
                if k + 1 < len(canon):
                    # software pipeline: next score before consuming this exp
                    pending = emit_score(*canon[k + 1])

                t = tpool.tile([P, le], F32, tag="t")
                nc.vector.tensor_tensor(out=t[:, lo:lo + ln], in0=ev, in1=sv,
                                        op=MULT)

                offs = [t0_off]
                if df != 0:
                    offs.append(t0_off - df)
                for to in offs:
                    tv = t[:, to:to + la]
                    if aidx == 0:
                        nc.scalar.copy(acc[:], tv)
                    else:
                        nc.vector.tensor_tensor(out=acc[:], in0=acc[:],
                                                in1=tv, op=ADD)
                    aidx += 1

                for to in offs:
                    eo = to + PAD
                    for ci in range(n_chunks):
                        mv = view2d(e[:], eo + ci * rpc * wp, rpc, w, wp)
                        nc.tensor.matmul(
                            psum[:, ci * mm_chunk:(ci + 1) * mm_chunk],
                            ident[:], mv,
                            start=(vidx == 0), stop=(vidx == n_views - 1),
                        )
                    vidx += 1

            den = fin.tile([P, lc], F32, tag="den")
            r = fin.tile([P, lc], F32, tag="r")
            xc = view2d(x[:], xq_off + PAD, br, w, wp)
            nc.vector.tensor_tensor(out=den[:], in0=psum[:], in1=xc, op=MULT)
            nc.vector.reciprocal_approx_fast(out=r[:], in_=den[:])
            out_c = fin.tile([P, lc], F32, tag="den")
            av = view2d(acc[:], PAD, br, w, wp)
            nc.vector.tensor_tensor(out=out_c[:], in0=av, in1=r[:], op=MULT)

            nc.sync.dma_start(out=y_out[:], in_=out_c[:])
    nc.compile()
    return nc


_NC_CACHE = {}


def _get_nc():
    if "nc" not in _NC_CACHE:
        _NC_CACHE["nc"] = build_nc()
    return _NC_CACHE["nc"]


def make_slabs(imgs, h=H, w=W):
    """[n,h,w] fp32 -> [n*4, 44*140] slab layout (p = rb*n + img)."""
    n = imgs.shape[0]
    br = h // RB_N
    slab = br + 2 * PAD
    xp = np.pad(imgs, ((0, 0), (PAD, PAD), (PAD, PAD)))
    rows = (np.arange(RB_N) * br)[:, None] + np.arange(slab)
    sl = xp[:, rows, :]
    sl = sl.transpose(1, 0, 2, 3)
    return np.ascontiguousarray(sl.reshape(RB_N * n, -1))


def unslab_out(y, n_img, h=H, w=W):
    """[n*4, br*w compact] -> [n, h, w]."""
    br = h // RB_N
    y = y.reshape(RB_N, n_img, br, w).transpose(1, 0, 2, 3)
    return np.ascontiguousarray(y.reshape(n_img, h, w))


def run(x, **spmd_kwargs):
    nc = _get_nc()
    imgs = np.ascontiguousarray(np.asarray(x).reshape(N_IMG_TOTAL, H, W))
    imgs = imgs.astype(np.float32, copy=False)
    ident = np.eye(128, dtype=np.float32)
    in_maps = [
        {"x": make_slabs(imgs[i * IMG_PER_CORE:(i + 1) * IMG_PER_CORE]),
         "ident": ident}
        for i in range(N_CORES)
    ]
    res = run_bass_kernel_spmd(nc, in_maps, core_ids=list(range(N_CORES)),
                               **spmd_kwargs)
    out = np.concatenate(
        [unslab_out(res.results[i]["y"], IMG_PER_CORE) for i in range(N_CORES)],
        axis=0,
    )
    return out.reshape(B, C, H, W).astype(np.float32, copy=False), res


def kernel(x):
    out, _ = run(x)
    return out


# revision 29
# speedup vs baseline: 1.0012x; 1.0012x over previous
"""Trainium2 Bass kernel for 7x7 sliding-window self-similarity attention.

out[b,c,h,w] = sum_j softmax_j(x[h,w] * x[h+dh,w+dw]) * x[h+dh,w+dw]
over the 7x7 neighborhood (zero padding, pad=3).

Sharding: B*C = 256 independent 128x128 images, 32 images per core on 8
NeuronCores (pure data parallel, no collectives).

Per-core layout: partition p = rowblock(0..3)*32 + image(0..31); each
partition holds a 44-row x 140-col zero-padded fp32 slab (6160 contiguous
floats), so every 7x7 shift is a flat offset view. Elementwise ops run on
fully contiguous 1D runs spanning the pad columns (finite garbage there,
never read).

Score symmetry: e_{-d}[i] == e_d[i-d]; only 25 canonical score tiles are
computed on an extended halo run; mirrored contributions are views.

Numerator trick: sum_d e_d[i]*x[i+d] = (sum of t_d = e_d*s_d views)/x[i]
(s_d is the score itself), so both the +d and -d numerator contributions
are views of one t tile; the final division by x cancels exactly:
out = acc_t / (x * sum_e).

Engines: DVE does score/t products and the acc_t chain; ACT does exp;
TensorE accumulates sum_e into PSUM via fp32 (LOW_HIGH) identity
matmuls on its own SBUF ports; GpSimd stays idle (it shares DVE's
second SBUF read port - concurrency measured 3x slower on both).
"""

import numpy as np

import concourse.bacc as bacc
import concourse.bass as bass  # noqa: F401
import concourse.tile as tile
from concourse import mybir
from concourse.bass_utils import run_bass_kernel_spmd

N_CORES = 8
F32 = mybir.dt.float32
MULT = mybir.AluOpType.mult
ADD = mybir.AluOpType.add

B, C, H, W = 4, 64, 128, 128
N_IMG_TOTAL = B * C
IMG_PER_CORE = N_IMG_TOTAL // N_CORES  # 32
RB_N = 4
PAD = 6
MM_CHUNK = 512                # one PSUM bank of fp32


def canonical_offsets():
    canon = [(0, 0)]
    canon += [(0, dj) for dj in range(1, 4)]
    canon += [(di, dj) for di in range(1, 4) for dj in range(-3, 4)]
    return canon


def view2d(ap, off, rows, cols, stride):
    """Strided [rows, cols] view at element offset `off` of a flat [P, L] AP."""
    a = ap.copy()
    pair_t = type(a.ap)
    part = list(a.ap)[0]
    a.ap = pair_t([list(part), [stride, rows], [1, cols]])
    a.offset = a.offset + off
    return a


def build_nc(n_img=IMG_PER_CORE, h=H, w=W):
    br = h // RB_N               # 32
    wp = w + 2 * PAD             # 140
    slab = br + 2 * PAD          # 44
    P = n_img * RB_N             # 128

    nx = slab * wp               # 6160
    le = (br + 6) * wp + 8       # 5328 extended run
    soff = 3 * wp - 4
    la = br * wp                 # 4480 full-width run
    lc = br * w                  # 4096 compact output
    t0_off = 3 * wp + 4
    xq_off = 6 * wp
    mm_chunk = min(MM_CHUNK, lc)
    n_chunks = lc // mm_chunk
    rpc = mm_chunk // w

    nc = bacc.Bacc("TRN2", target_bir_lowering=False, debug=False)
    x_in = nc.dram_tensor("x", [P, nx], F32, kind="ExternalInput")
    id_in = nc.dram_tensor("ident", [P, P], F32, kind="ExternalInput")
    y_out = nc.dram_tensor("y", [P, lc], F32, kind="ExternalOutput")

    canon = canonical_offsets()
    n_views = 2 * len(canon) - 1  # 49

    with tile.TileContext(nc) as tc:
        with (
            tc.tile_pool(name="big", bufs=1) as big,
            tc.tile_pool(name="sp", bufs=2) as spool,
            tc.tile_pool(name="ep", bufs=2) as epool,
            tc.tile_pool(name="tp", bufs=1) as tpool,
            tc.tile_pool(name="fin", bufs=1) as fin,
            tc.tile_pool(name="ps", bufs=1, space="PSUM") as ps,
        ):
            x = big.tile([P, nx], F32, tag="x")
            ident = big.tile([P, P], F32, tag="id")
            acc = big.tile([P, la], F32, tag="acc")
            psum = ps.tile([P, lc], F32, tag="sum")

            nrd = 6 * wp + la + 3 * wp + 3 + 1  # last element ever read
            nc.sync.dma_start(out=x[:, :nrd], in_=x_in[:, :nrd])
            nc.sync.dma_start(out=ident[:], in_=id_in[:])

            vidx = 0
            aidx = 0

            def emit_score(di, dj):
                # s_d = x * shift(x, d) on the minimal run [t0-df, t0+la)
                df = di * wp + dj
                lo = t0_off - df
                ln = la + df
                s = spool.tile([P, le], F32, tag="s")
                e = epool.tile([P, le], F32, tag="e")
                sv = s[:, lo:lo + ln]
                if df == 0:
                    nc.scalar.activation(
                        out=sv, in_=x[:, soff + lo:soff + lo + ln],
                        func=mybir.ActivationFunctionType.Square,
                    )
                else:
                    nc.vector.tensor_tensor(
                        out=sv,
                        in0=x[:, soff + lo:soff + lo + ln],
                        in1=x[:, soff + lo + df:soff + lo + df + ln],
                        op=MULT,
                    )
                ev = e[:, lo:lo + ln]
                nc.scalar.activation(
                    out=ev, in_=sv, func=mybir.ActivationFunctionType.Exp
                )
                return s, e, sv, ev, df, lo, ln

            pending = emit_score(*canon[0])
            for k in range(len(canon)):
                s, e, sv, ev, df, lo, ln = pending
                if k + 1 < len(canon):
                    # software pipeline: next score before consuming this exp
                    pending = emit_score(*canon[k + 1])

                t = tpool.tile([P, le], F32, tag="t")
                nc.vector.tensor_tensor(out=t[:, lo:lo + ln], in0=ev, in1=sv,
                                        op=MULT)

                offs = [t0_off]
                if df != 0:
                    offs.append(t0_off - df)
                for to in offs:
                    tv = t[:, to:to + la]
                    if aidx == 0:
                        nc.scalar.copy(acc[:], tv)
                    else:
                        nc.vector.tensor_tensor(out=acc[:], in0=acc[:],
                                                in1=tv, op=ADD)
                    aidx += 1

                for to in offs:
                    eo = to + PAD
                    for ci in range(n_chunks):
                        mv = view2d(e[:], eo + ci * rpc * wp, rpc, w, wp)
                        nc.tensor.matmul(
                            psum[:, ci * mm_chunk:(ci + 1) * mm_chunk],
                            ident[:], mv,
                            start=(vidx == 0), stop=(vidx == n_views - 1),
                        )
                    vidx += 1

            den = fin.tile([P, lc], F32, tag="den")
            r = fin.tile([P, lc], F32, tag="r")
            xc = view2d(x[:], xq_off + PAD, br, w, wp)
            nc.vector.tensor_tensor(out=den[:], in0=psum[:], in1=xc, op=MULT)
            nc.vector.reciprocal_approx_fast(out=r[:], in_=den[:])
            out_c = fin.tile([P, lc], F32, tag="den")
            av = view2d(acc[:], PAD, br, w, wp)
            nc.vector.tensor_tensor(out=out_c[:], in0=av, in1=r[:], op=MULT)

            nc.sync.dma_start(out=y_out[:], in_=out_c[:])
    nc.compile()
    return nc


_NC_CACHE = {}


def _get_nc():
    if "nc" not in _NC_CACHE:
        _NC_CACHE["nc"] = build_nc()
    return _NC_CACHE["nc"]


def make_slabs(imgs, h=H, w=W):
    """[n,h,w] fp32 -> [n*4, 44*140] slab layout (p = rb*n + img)."""
    n = imgs.shape[0]
    br = h // RB_N
    slab = br + 2 * PAD
    xp = np.pad(imgs, ((0, 0), (PAD, PAD), (PAD, PAD)))
    rows = (np.arange(RB_N) * br)[:, None] + np.arange(slab)
    sl = xp[:, rows, :]
    sl = sl.transpose(1, 0, 2, 3)
    return np.ascontiguousarray(sl.reshape(RB_N * n, -1))


def unslab_out(y, n_img, h=H, w=W):
    """[n*4, br*w compact] -> [n, h, w]."""
    br = h // RB_N
    y = y.reshape(RB_N, n_img, br, w).transpose(1, 0, 2, 3)
    return np.ascontiguousarray(y.reshape(n_img, h, w))


def run(x, **spmd_kwargs):
    nc = _get_nc()
    imgs = np.ascontiguousarray(np.asarray(x).reshape(N_IMG_TOTAL, H, W))
    imgs = imgs.astype(np.float32, copy=False)
    ident = np.eye(128, dtype=np.float32)
    in_maps = [
        {"x": make_slabs(imgs[i * IMG_PER_CORE:(i + 1) * IMG_PER_CORE]),
         "ident": ident}
        for i in range(N_CORES)
    ]
    res = run_bass_kernel_spmd(nc, in_maps, core_ids=list(range(N_CORES)),
                               **spmd_kwargs)
    out = np.concatenate(
        [unslab_out(res.results[i]["y"], IMG_PER_CORE) for i in range(N_CORES)],
        axis=0,
    )
    return out.reshape(B, C, H, W).astype(np.float32, copy=False), res


def kernel(x):
    out, _ = run(x)
    return out
